# revision 15
# baseline (speedup 1.0000x reference)
"""ConvMod kernel for Trainium2 (8 NeuronCores, batch-parallel).

Per-sample modulated 3x3 grouped conv:
  style = w @ (fce_kernel*fce_scale) + fce_bias                [B, CIN]
  wp    = conv_kernel * conv_scale * style                     [B,3,3,CIN,NF]
  wpp   = wp * rsqrt(sum(wp^2, (ky,kx,cin)) + 1e-8)            demodulated
  out   = conv2d_same(x, wpp per-sample) + conv_bias           [B,H,W,NF]

Sharding: batch B=8 across 8 cores (1 sample/core), params replicated.

Device layout: M=128 matmul packing. PSUM partitions hold (2 output rows x
64 channels). The x tile duplicates channels on partitions 64-127 shifted
by +2 rows, so one K=128 matmul contracts two input rows at once with a
block-structured lhsT. Per 4 output rows (supergroup): 6 matmuls of
free-size 512 cover all 9 taps for all 4 rows.

Key scheduling facts this file is built around (TimelineSim cost model):
 - demod scales only the output channel, so conv(x, wp*diag(wstd)) =
   conv(x, wp)*wstd[n]: wstd is applied as a per-partition scale in the
   PSUM->SBUF drain (fused with the conv_bias add), keeping the sqrt
   chain off the critical path. ssum[n] = sum_c stylec[c]^2 *
   (sum_t ck[c,t,n]^2) collapses the reduction to one tiny matmul whose
   [64,1] result lands in drain-scale layout directly.
 - FCE_SCALE*CONV_SCALE is folded into host-prescaled fce_k, and
   fce_bias*CONV_SCALE enters as a 5th accumulating K=1 matmul, so
   stylec is a plain PSUM->SBUF copy (no ACT hop, no separate bias DMA
   on the critical path).
 - the PE p-state ramp needs ~3us of continuous execution to reach
   2.4GHz and resets on multi-us idle gaps; dummy warm-up matmuls
   bracket the style matmuls and keep PE busy until the first conv
   matmul's inputs land, so the conv runs at full clock throughout.
 - per-DMA fixed costs (SEQ 565/HWDGE 625/delay 650/sem 900ns) dominate
   small transfers: weights are host-repacked contiguous f16; wv /
   fce_b / conv_bias ride the Pool SWDGE generator (parallel to HWDGE).
 - drains for sg0..2 are emitted only after the wstd chain (ssum at
   sg2 in the PE stream, sroot before any ACT drain, recip/wstd2
   before any DVE drain) -- ordering, not timing, prevents deadlock.
 - x and conv weights travel as f16 (tolerance 2e-2 >> f16 rounding);
   f16 matmul rate equals f32r on TRN2 (1 col/cycle).
 - tail: the last supergroup drains as two halves on ACT+DVE in
   parallel and stores via the idle Pool SWDGE path (lower fixed
   latency than ACT/HWDGE).
"""

import numpy as np

B, H, W, CIN = 8, 256, 256, 64
WDIM, NF, KK = 512, 64, 3
NCORES = 8
CR = 32                 # output rows per x chunk
NCH = H // CR           # 8 chunks
SGC = CR // 4           # supergroups (4 output rows) per chunk
NSG = NCH * SGC
JW = WDIM // 128
FCE_SCALE = float(np.sqrt(1.0 / WDIM))
CONV_SCALE = float(np.sqrt(1.0 / 0.6 / (KK * KK * CIN)))
NWARM_A = 24            # warm-up matmuls before the style matmuls
NWARM_B = 20            # warm-up matmuls after the style matmuls

_CACHE = {}


def _build(repeats=1):
    import concourse.mybir as mybir
    import concourse.tile as tile
    from concourse import bacc

    f32 = mybir.dt.float32
    f16 = mybir.dt.float16
    nc = bacc.Bacc("TRN2", target_bir_lowering=False, debug=False,
                   num_devices=NCORES)

    # weight tensors host-repacked into SBUF layouts, f16, prescaled
    xt = nc.dram_tensor("xt", [CIN, H, W], f16, kind="ExternalInput").ap()
    wv = nc.dram_tensor("wv", [128, JW + 1], f16,
                        kind="ExternalInput").ap()
    fce_k = nc.dram_tensor("fce_k", [128, JW + 1, CIN], f16,
                           kind="ExternalInput").ap()
    ck_d = nc.dram_tensor("ck", [CIN, KK * KK, NF], f16,
                          kind="ExternalInput").ap()
    cb_d = nc.dram_tensor("cb", [NF], f32, kind="ExternalInput").ap()
    # out: partition p = ro*64 + n (ro = row parity), free = (g, col) with
    # output row = 2g + ro
    ytd = nc.dram_tensor("ytd", [2 * NF, (H // 2) * W], f16,
                         kind="ExternalOutput").ap()

    NT = KK * KK  # 9 taps
    # chunk 0 starts extra fine so the first supergroup's band is ready
    # early; chunk 1 fine; later chunks coarse (fewer sync instructions)
    SEGS = {0: [0, 5, 10, 18, 26, CR + 2], 1: [0, 10, 18, 26, CR + 2]}
    BANDS = {0: [0, 3, 8, 16, 24, CR], 1: [0, 8, 16, 24, CR]}
    SEGS_C = [0, 18, CR + 2]
    BANDS_C = [0, 16, CR]

    def nseg(cj):
        return len(SEGS.get(cj, SEGS_C)) - 1

    with tile.TileContext(nc) as tc:
        with (
            tc.tile_pool(name="const", bufs=1) as const,
            tc.tile_pool(name="prep", bufs=1) as prep,
            tc.tile_pool(name="pps", bufs=1, space="PSUM") as pps,
            tc.tile_pool(name="wrmp", bufs=1, space="PSUM") as wrmp,
            tc.tile_pool(name="xin", bufs=1) as xin,
            tc.tile_pool(name="yout", bufs=4) as yout,
            tc.tile_pool(name="acc", bufs=6, space="PSUM") as accp,
        ):
            # dummy Sqrt first on ACT: hoists the activation-table load
            dmy = const.tile([1, 1], f32)
            nc.vector.memset(dmy, 1.0)
            dmy2 = prep.tile([1, 1], f32)
            nc.scalar.sqrt(dmy2, dmy)

            # warm-up operands + the ones-rhs for the fce_b matmul
            # (DVE memsets: Pool must reach the wv SWDGE gen immediately)
            wrm_l = const.tile([1, 1], f16)
            nc.vector.memset(wrm_l, 0.0)
            wrm_r = const.tile([1, 128], f16)
            nc.vector.memset(wrm_r, 0.0)

            # SP/HWDGE: fce (gates style), first x segment (gates first
            # band), ck (gates L build + ckk), rest of the x segments.
            # Pool/SWDGE (parallel generator): wv, fb2, conv_bias.
            fce_sb = prep.tile([128, JW + 1, CIN], f16)
            nc.sync.dma_start(out=fce_sb, in_=fce_k)

            xxb = [xin.tile([2 * CIN, CR + 2, W + 2], f16, name=f"xx{k}")
                   for k in range(4)]

            def emit_load_seg(cj, si):
                l0 = 1 if cj == 0 else 2
                l1 = CR + 1 if cj == NCH - 1 else CR + 2
                R0 = cj * CR
                segs = SEGS.get(cj, SEGS_C)
                a = max(segs[si], l0)
                b = min(segs[si + 1], l1)
                nc.sync.dma_start(
                    out=xxb[cj % 4][0:CIN, a:b, 1:W + 1],
                    in_=xt[:, R0 - 1 + a:R0 - 1 + b, :])

            for si in range(nseg(0)):
                emit_load_seg(0, si)
            for si in range(nseg(1)):
                emit_load_seg(1, si)

            wv_sb = prep.tile([128, JW + 1], f16)
            nc.gpsimd.dma_start(out=wv_sb, in_=wv)
            ck_sb = prep.tile([CIN, NT, NF], f16)
            nc.gpsimd.dma_start(out=ck_sb, in_=ck_d)

            # L zero quadrants early on Pool, conv_bias after
            L1 = const.tile([2 * CIN, KK, 2 * NF], f16)
            L2 = const.tile([2 * CIN, KK, 2 * NF], f16)
            nc.gpsimd.memset(L1[0:CIN, :, NF:2 * NF], 0.0)
            nc.gpsimd.memset(L2[CIN:2 * CIN, :, 0:NF], 0.0)
            cb2_sb = const.tile([2 * NF, 1], f32)
            nc.gpsimd.dma_start(out=cb2_sb[0:NF, :], in_=cb_d)
            nc.gpsimd.dma_start(out=cb2_sb[NF:2 * NF, :], in_=cb_d)

            # zero row for padding writes + x-tile column borders
            zrow = const.tile([CIN, 1, W + 2], f16)
            nc.vector.memset(zrow.rearrange("c a w -> c (a w)"), 0.0)
            zcol = zrow[:, 0:1, 0:CR + 2].rearrange("c a w -> c w a")
            nc.vector.tensor_copy(xxb[0][0:CIN, :, 0:1], zcol)
            nc.vector.tensor_copy(xxb[0][0:CIN, :, W + 1:W + 2], zcol)
            # remaining x-tile borders off the critical DVE queue (Pool;
            # first needed when chunk 1 computes, ~14us in)
            for k in range(1, 4):
                nc.gpsimd.tensor_copy(xxb[k][0:CIN, :, 0:1], zcol)
                nc.gpsimd.tensor_copy(xxb[k][0:CIN, :, W + 1:W + 2], zcol)

            def emit_band(cj, si):
                xx = xxb[cj % 4]
                if si == 0:
                    if cj == 0:
                        nc.vector.tensor_copy(xx[0:CIN, 0:1, :], zrow)
                    else:
                        nc.vector.tensor_copy(
                            xx[0:CIN, 0:2, :],
                            xxb[(cj - 1) % 4][0:CIN, CR:CR + 2, :])
                bands = BANDS.get(cj, BANDS_C)
                if si == nseg(cj) - 1 and cj == NCH - 1:
                    nc.vector.tensor_copy(xx[0:CIN, CR + 1:CR + 2, :], zrow)
                ba, bb = bands[si], bands[si + 1]
                nc.vector.tensor_copy(xx[CIN:2 * CIN, ba:bb, :],
                                      xx[0:CIN, ba + 2:bb + 2, :])

            # PE warm-up bracket around the style matmuls
            ppsbig = pps.tile([128, 512], f32)
            # warm-ups get their own PSUM bank: PSUM dependency tracking
            # is tile-granular, so sharing ppsbig would make stylec wait
            # for the B-warmups
            wrm_tile = wrmp.tile([128, 512], f32, name="wrmps")
            wrm_ps = wrm_tile[0:1, 0:128]
            for _w in range(NWARM_A):
                nc.tensor.matmul(wrm_ps, lhsT=wrm_l, rhs=wrm_r,
                                 start=True, stop=True)
            # style_ps = sum_j fce_j^T wv_j + fb2 (scales pre-folded)
            # style_ps = sum_j fce_j^T wv_j; slot j=JW carries fce_bias
            # via a one-hot wv column (host-packed)
            style_ps = ppsbig[0:CIN, 0:1]
            for j in range(JW + 1):
                nc.tensor.matmul(style_ps, lhsT=fce_sb[:, j, :],
                                 rhs=wv_sb[:, j:j + 1],
                                 start=(j == 0), stop=(j == JW))
            for _w in range(NWARM_B):
                nc.tensor.matmul(wrm_ps, lhsT=wrm_l, rhs=wrm_r,
                                 start=True, stop=True)

            stylec = prep.tile([CIN, 1], f32)
            nc.vector.tensor_copy(stylec, style_ps)
            emit_band(0, 0)

            # Block-structured lhsT tiles, straight from ck * stylec (no
            # demod mul -- applied at drain time).
            # lhsT[k, m]: k<64 = channels of x row XA, k>=64 = x row XA+2;
            # m<64 = out row r (ro=0) channels, m>=64 = out row r+1 (ro=1).
            # mm1 (XA = r-1): (A,ro0)=w[-1,s-1] (B,ro0)=w[+1,s-1]
            #                 (B,ro1)=w[0,s-1]  (A,ro1)=0
            # mm2 (XA = r):   (A,ro0)=w[0,s-1]  (A,ro1)=w[-1,s-1]
            #                 (B,ro1)=w[+1,s-1] (B,ro0)=0
            # wp tap index t = (dy+1)*3 + (dx+1), dx = s-1.
            def lmul(dst, t0):
                nc.vector.tensor_scalar_mul(dst, ck_sb[:, t0:t0 + 3, :],
                                            stylec)

            lmul(L1[0:CIN, :, 0:NF], 0)
            lmul(L1[CIN:2 * CIN, :, 0:NF], 6)
            lmul(L1[CIN:2 * CIN, :, NF:2 * NF], 3)
            lmul(L2[0:CIN, :, 0:NF], 3)
            lmul(L2[0:CIN, :, NF:2 * NF], 0)
            lmul(L2[CIN:2 * CIN, :, NF:2 * NF], 6)

            emit_band(0, 1)
            emit_band(0, 2)

            # ckk[c,n] = sum_t ck^2 on DVE; ssum/sroot/recip/wstd2 are
            # emitted inside the sg2 iteration (see loop) so the PE
            # stream never stalls on them.
            sq = prep.tile([CIN, NT, NF], f32)
            nc.vector.tensor_mul(sq.rearrange("c t n -> c (t n)"),
                                 ck_sb.rearrange("c t n -> c (t n)"),
                                 ck_sb.rearrange("c t n -> c (t n)"))
            t4 = prep.tile([CIN, 4, NF], f32)
            nc.vector.tensor_add(t4.rearrange("c t n -> c (t n)"),
                                 sq[:, 0:4, :].rearrange("c t n -> c (t n)"),
                                 sq[:, 4:8, :].rearrange("c t n -> c (t n)"))
            t2 = prep.tile([CIN, 2, NF], f32)
            nc.vector.tensor_add(t2.rearrange("c t n -> c (t n)"),
                                 t4[:, 0:2, :].rearrange("c t n -> c (t n)"),
                                 t4[:, 2:4, :].rearrange("c t n -> c (t n)"))
            ckk = prep.tile([CIN, NF], f32)
            nc.vector.tensor_add(ckk, t2[:, 0, :], t2[:, 1, :])
            nc.vector.tensor_add(ckk, ckk, sq[:, 8, :])
            stylec2 = prep.tile([CIN, 1], f32)
            nc.vector.tensor_mul(stylec2, stylec, stylec)
            eps_sb = prep.tile([CIN, 1], f32)
            nc.vector.memset(eps_sb, 1e-8)

            emit_band(0, 3)
            emit_band(0, 4)

            ssum_ps = ppsbig[0:CIN, 64:65]
            sroot = prep.tile([CIN, 1], f32)
            wstdT = prep.tile([CIN, 1], f32)
            wstd2 = const.tile([2 * NF, 1], f32)

            def emit_wstd():
                nc.tensor.matmul(ssum_ps, lhsT=ckk, rhs=stylec2,
                                 start=True, stop=True)
                nc.scalar.activation(sroot, ssum_ps,
                                     mybir.ActivationFunctionType.Sqrt,
                                     bias=eps_sb, scale=1.0)
                nc.vector.reciprocal(wstdT, sroot)
                nc.vector.tensor_copy(wstd2[0:NF, :], wstdT)
                nc.vector.tensor_copy(wstd2[NF:2 * NF, :], wstdT)

            def drain(sg, ps, yslice):
                # y = ps*wstd[n] + cb[n]; parity alternates ACT/DVE
                if sg % 2 == 0:
                    nc.scalar.activation(
                        yslice, ps,
                        mybir.ActivationFunctionType.Identity,
                        bias=cb2_sb, scale=wstd2)
                else:
                    nc.vector.tensor_scalar(
                        yslice, ps, wstd2, cb2_sb,
                        op0=mybir.AluOpType.mult,
                        op1=mybir.AluOpType.add)

            # ---- main conv loop (software-pipelined emission) ----
            # Chunk ci+1's segment loads and dup bands are emitted inside
            # chunk ci's supergroup loop so every engine queue interleaves
            # producer work for the next chunk with consumer work for the
            # current one.
            first = True
            for _ in range(repeats):
                if not first:
                    for si in range(nseg(0)):
                        emit_load_seg(0, si)
                    for si in range(nseg(1)):
                        emit_load_seg(1, si)
                    for si in range(nseg(0)):
                        emit_band(0, si)
                pend = []          # (sg, ps, yslice) deferred for sg0..2
                ys = None
                for ci in range(NCH):
                    xx = xxb[ci % 4]
                    xxr = xx.rearrange("p (a b) w -> p b a w", b=2)
                    for q in range(SGC):
                        if q == 0 and ci + 2 < NCH:
                            for si in range(nseg(ci + 2)):
                                emit_load_seg(ci + 2, si)
                        if ci + 1 < NCH and q % 2 == 1:
                            si = (q - 1) // 2
                            if si < nseg(ci + 1):
                                emit_band(ci + 1, si)
                        sg = ci * SGC + q
                        k2 = sg % 2
                        if k2 == 0:
                            ys = yout.tile([2 * NF, 2 * 2 * W], f16)
                        ps = accp.tile([2 * NF, 2 * W], f32)
                        for s in range(KK):
                            nc.tensor.matmul(
                                ps, lhsT=L1[:, s, :],
                                rhs=xxr[:, 0, 2 * q:2 * q + 2, s:s + W],
                                start=(s == 0), stop=False)
                        for s in range(KK):
                            nc.tensor.matmul(
                                ps, lhsT=L2[:, s, :],
                                rhs=xxr[:, 1, 2 * q:2 * q + 2, s:s + W],
                                start=False, stop=(s == KK - 1))
                        yslice = ys[:, k2 * 2 * W:(k2 + 1) * 2 * W]
                        if first and sg < 2:
                            pend.append((sg, ps, yslice, ys))
                            continue
                        if first and sg == 2:
                            emit_wstd()
                            for (psg, p_ps, p_ysl, p_ys) in pend:
                                drain(psg, p_ps, p_ysl)
                            nc.scalar.dma_start(
                                out=ytd[:, 0:4 * W], in_=pend[1][3])
                            pend = []
                        drain(sg, ps, yslice)
                        if sg >= NSG - 2:
                            # last two supergroups store individually so
                            # the final store is as small/early as possible
                            nc.scalar.dma_start(
                                out=ytd[:, sg * 2 * W:(sg + 1) * 2 * W],
                                in_=yslice)
                        elif k2 == 1:
                            nc.scalar.dma_start(
                                out=ytd[:, (sg - 1) * 2 * W:(sg + 1) * 2 * W],
                                in_=ys)
                first = False

    nc.compile()
    return nc


def _get(repeats=1):
    if repeats not in _CACHE:
        _CACHE[repeats] = _build(repeats)
    return _CACHE[repeats]


def _wv_aug(w_b):
    # [128, JW+1]: col JW is the one-hot selecting the bias row
    out = np.zeros((128, JW + 1), np.float32)
    out[:, :JW] = np.asarray(w_b, np.float32).reshape(JW, 128).T
    out[0, JW] = 1.0
    return out


def _fce_aug(fce_kernel, fce_bias):
    # [128, JW+1, CIN]: slots 0..JW-1 = fce_kernel*FCE_SCALE*CONV_SCALE;
    # slot JW partition 0 = fce_bias*CONV_SCALE (selected by the one-hot)
    out = np.zeros((128, JW + 1, CIN), np.float32)
    out[:, :JW, :] = (np.asarray(fce_kernel, np.float32)
                      * (FCE_SCALE * CONV_SCALE)).reshape(
                          JW, 128, CIN).transpose(1, 0, 2)
    out[0, JW, :] = np.asarray(fce_bias, np.float32) * CONV_SCALE
    return np.ascontiguousarray(out)


def _pack(x_b, w_b, fce_kernel, fce_bias, conv_kernel, conv_bias):
    f16 = np.float16
    return {
        "xt": np.ascontiguousarray(
            np.asarray(x_b, np.float32).transpose(2, 0, 1)).astype(f16),
        "wv": _wv_aug(w_b).astype(f16),
        "fce_k": _fce_aug(fce_kernel, fce_bias).astype(f16),
        "ck": np.ascontiguousarray(
            np.asarray(conv_kernel, np.float32)
            .transpose(2, 0, 1, 3).reshape(CIN, KK * KK, NF)).astype(f16),
        "cb": np.asarray(conv_bias, np.float32),
    }


def kernel(x, w, fce_kernel, fce_bias, conv_kernel, conv_bias):
    from concourse.bass_utils import run_bass_kernel_spmd

    nc = _get()
    in_maps = [_pack(x[b], w[b], fce_kernel, fce_bias,
                     conv_kernel, conv_bias) for b in range(B)]
    res = run_bass_kernel_spmd(nc, in_maps, core_ids=list(range(NCORES)))
    out = np.empty((B, H, W, NF), np.float32)
    for b in range(B):
        a = np.asarray(res.results[b]["ytd"]).astype(np.float32)
        # [ro*64+n, g*W+col] -> [h, w, n] with h = 2g + ro
        a = a.reshape(2, NF, H // 2, W).transpose(2, 0, 3, 1)
        out[b] = a.reshape(H, W, NF)
    return out


# revision 16
# speedup vs baseline: 1.0084x; 1.0084x over previous
"""ConvMod kernel for Trainium2 (8 NeuronCores, batch-parallel).

Per-sample modulated 3x3 grouped conv:
  style = w @ (fce_kernel*fce_scale) + fce_bias                [B, CIN]
  wp    = conv_kernel * conv_scale * style                     [B,3,3,CIN,NF]
  wpp   = wp * rsqrt(sum(wp^2, (ky,kx,cin)) + 1e-8)            demodulated
  out   = conv2d_same(x, wpp per-sample) + conv_bias           [B,H,W,NF]

Sharding: batch B=8 across 8 cores (1 sample/core), params replicated.

Device layout: M=128 matmul packing. PSUM partitions hold (2 output rows x
64 channels). The x tile duplicates channels on partitions 64-127 shifted
by +2 rows, so one K=128 matmul contracts two input rows at once with a
block-structured lhsT. Per 4 output rows (supergroup): 6 matmuls of
free-size 512 cover all 9 taps for all 4 rows.

Key scheduling facts this file is built around (TimelineSim cost model):
 - demod scales only the output channel, so conv(x, wp*diag(wstd)) =
   conv(x, wp)*wstd[n]: wstd is applied as a per-partition scale in the
   PSUM->SBUF drain (fused with the conv_bias add), keeping the sqrt
   chain off the critical path. ssum[n] = sum_c stylec[c]^2 *
   (sum_t ck[c,t,n]^2) collapses the reduction to one tiny matmul whose
   [64,1] result lands in drain-scale layout directly.
 - FCE_SCALE*CONV_SCALE is folded into host-prescaled fce_k, and
   fce_bias*CONV_SCALE enters as a 5th accumulating K=1 matmul, so
   stylec is a plain PSUM->SBUF copy (no ACT hop, no separate bias DMA
   on the critical path).
 - the PE p-state ramp needs ~3us of continuous execution to reach
   2.4GHz and resets on multi-us idle gaps; dummy warm-up matmuls
   bracket the style matmuls and keep PE busy until the first conv
   matmul's inputs land, so the conv runs at full clock throughout.
 - per-DMA fixed costs (SEQ 565/HWDGE 625/delay 650/sem 900ns) dominate
   small transfers: weights are host-repacked contiguous f16; wv /
   fce_b / conv_bias ride the Pool SWDGE generator (parallel to HWDGE).
 - drains for sg0..2 are emitted only after the wstd chain (ssum at
   sg2 in the PE stream, sroot before any ACT drain, recip/wstd2
   before any DVE drain) -- ordering, not timing, prevents deadlock.
 - x and conv weights travel as f16 (tolerance 2e-2 >> f16 rounding);
   f16 matmul rate equals f32r on TRN2 (1 col/cycle).
 - tail: the last supergroup drains as two halves on ACT+DVE in
   parallel and stores via the idle Pool SWDGE path (lower fixed
   latency than ACT/HWDGE).
"""

import numpy as np

B, H, W, CIN = 8, 256, 256, 64
WDIM, NF, KK = 512, 64, 3
NCORES = 8
CR = 32                 # output rows per x chunk
NCH = H // CR           # 8 chunks
SGC = CR // 4           # supergroups (4 output rows) per chunk
NSG = NCH * SGC
JW = WDIM // 128
FCE_SCALE = float(np.sqrt(1.0 / WDIM))
CONV_SCALE = float(np.sqrt(1.0 / 0.6 / (KK * KK * CIN)))
NWARM_A = 19            # warm-up matmuls before the style matmuls
NWARM_B = 28            # warm-up matmuls after the style matmuls

_CACHE = {}


def _build(repeats=1):
    import concourse.mybir as mybir
    import concourse.tile as tile
    from concourse import bacc

    f32 = mybir.dt.float32
    f16 = mybir.dt.float16
    nc = bacc.Bacc("TRN2", target_bir_lowering=False, debug=False,
                   num_devices=NCORES)

    # weight tensors host-repacked into SBUF layouts, f16, prescaled
    xt = nc.dram_tensor("xt", [CIN, H, W], f16, kind="ExternalInput").ap()
    wv = nc.dram_tensor("wv", [128, JW + 1], f16,
                        kind="ExternalInput").ap()
    fce_k = nc.dram_tensor("fce_k", [128, JW + 1, CIN], f16,
                           kind="ExternalInput").ap()
    ck_d = nc.dram_tensor("ck", [CIN, KK * KK, NF], f16,
                          kind="ExternalInput").ap()
    cb_d = nc.dram_tensor("cb", [NF], f32, kind="ExternalInput").ap()
    # out: partition p = ro*64 + n (ro = row parity), free = (g, col) with
    # output row = 2g + ro
    ytd = nc.dram_tensor("ytd", [2 * NF, (H // 2) * W], f16,
                         kind="ExternalOutput").ap()

    NT = KK * KK  # 9 taps
    # chunk 0 starts extra fine so the first supergroup's band is ready
    # early; chunk 1 fine; later chunks coarse (fewer sync instructions)
    SEGS = {0: [0, 5, 10, 18, 26, CR + 2], 1: [0, 10, 18, 26, CR + 2]}
    BANDS = {0: [0, 3, 8, 16, 24, CR], 1: [0, 8, 16, 24, CR]}
    SEGS_C = [0, 18, CR + 2]
    BANDS_C = [0, 16, CR]

    def nseg(cj):
        return len(SEGS.get(cj, SEGS_C)) - 1

    with tile.TileContext(nc) as tc:
        with (
            tc.tile_pool(name="const", bufs=1) as const,
            tc.tile_pool(name="prep", bufs=1) as prep,
            tc.tile_pool(name="pps", bufs=1, space="PSUM") as pps,
            tc.tile_pool(name="wrmp", bufs=1, space="PSUM") as wrmp,
            tc.tile_pool(name="xin", bufs=1) as xin,
            tc.tile_pool(name="yout", bufs=4) as yout,
            tc.tile_pool(name="acc", bufs=6, space="PSUM") as accp,
        ):
            # dummy Sqrt first on ACT: hoists the activation-table load
            dmy = const.tile([1, 1], f32)
            nc.vector.memset(dmy, 1.0)
            dmy2 = prep.tile([1, 1], f32)
            nc.scalar.sqrt(dmy2, dmy)

            # warm-up operands + the ones-rhs for the fce_b matmul
            # (DVE memsets: Pool must reach the wv SWDGE gen immediately)
            wrm_l = const.tile([1, 1], f16)
            nc.vector.memset(wrm_l, 0.0)
            wrm_r = const.tile([1, 128], f16)
            nc.vector.memset(wrm_r, 0.0)

            # SP/HWDGE: fce (gates style), first x segment (gates first
            # band), ck (gates L build + ckk), rest of the x segments.
            # Pool/SWDGE (parallel generator): wv, fb2, conv_bias.
            fce_sb = prep.tile([128, JW + 1, CIN], f16)
            nc.sync.dma_start(out=fce_sb, in_=fce_k)

            xxb = [xin.tile([2 * CIN, CR + 2, W + 2], f16, name=f"xx{k}")
                   for k in range(4)]

            def emit_load_seg(cj, si):
                l0 = 1 if cj == 0 else 2
                l1 = CR + 1 if cj == NCH - 1 else CR + 2
                R0 = cj * CR
                segs = SEGS.get(cj, SEGS_C)
                a = max(segs[si], l0)
                b = min(segs[si + 1], l1)
                nc.sync.dma_start(
                    out=xxb[cj % 4][0:CIN, a:b, 1:W + 1],
                    in_=xt[:, R0 - 1 + a:R0 - 1 + b, :])

            emit_load_seg(0, 0)
            ck_sb = prep.tile([CIN, NT, NF], f16)
            nc.sync.dma_start(out=ck_sb, in_=ck_d)
            for si in range(1, nseg(0)):
                emit_load_seg(0, si)
            for si in range(nseg(1)):
                emit_load_seg(1, si)

            wv_sb = prep.tile([128, JW + 1], f16)
            nc.gpsimd.dma_start(out=wv_sb, in_=wv)

            # L zero quadrants early on Pool, conv_bias after
            L1 = const.tile([2 * CIN, KK, 2 * NF], f16)
            L2 = const.tile([2 * CIN, KK, 2 * NF], f16)
            nc.gpsimd.memset(L1[0:CIN, :, NF:2 * NF], 0.0)
            nc.gpsimd.memset(L2[CIN:2 * CIN, :, 0:NF], 0.0)
            cb2_sb = const.tile([2 * NF, 1], f32)
            nc.gpsimd.dma_start(out=cb2_sb[0:NF, :], in_=cb_d)
            nc.gpsimd.dma_start(out=cb2_sb[NF:2 * NF, :], in_=cb_d)

            # zero row for padding writes + x-tile column borders
            zrow = const.tile([CIN, 1, W + 2], f16)
            nc.vector.memset(zrow.rearrange("c a w -> c (a w)"), 0.0)
            zcol = zrow[:, 0:1, 0:CR + 2].rearrange("c a w -> c w a")
            nc.vector.tensor_copy(xxb[0][0:CIN, :, 0:1], zcol)
            nc.vector.tensor_copy(xxb[0][0:CIN, :, W + 1:W + 2], zcol)
            # remaining x-tile borders off the critical DVE queue (Pool;
            # first needed when chunk 1 computes, ~14us in)
            for k in range(1, 4):
                nc.gpsimd.tensor_copy(xxb[k][0:CIN, :, 0:1], zcol)
                nc.gpsimd.tensor_copy(xxb[k][0:CIN, :, W + 1:W + 2], zcol)

            def emit_band(cj, si):
                xx = xxb[cj % 4]
                if si == 0:
                    if cj == 0:
                        nc.vector.tensor_copy(xx[0:CIN, 0:1, :], zrow)
                    else:
                        nc.vector.tensor_copy(
                            xx[0:CIN, 0:2, :],
                            xxb[(cj - 1) % 4][0:CIN, CR:CR + 2, :])
                bands = BANDS.get(cj, BANDS_C)
                if si == nseg(cj) - 1 and cj == NCH - 1:
                    nc.vector.tensor_copy(xx[0:CIN, CR + 1:CR + 2, :], zrow)
                ba, bb = bands[si], bands[si + 1]
                nc.vector.tensor_copy(xx[CIN:2 * CIN, ba:bb, :],
                                      xx[0:CIN, ba + 2:bb + 2, :])

            # PE warm-up bracket around the style matmuls
            ppsbig = pps.tile([128, 512], f32)
            # warm-ups get their own PSUM bank: PSUM dependency tracking
            # is tile-granular, so sharing ppsbig would make stylec wait
            # for the B-warmups
            wrm_tile = wrmp.tile([128, 512], f32, name="wrmps")
            wrm_ps = wrm_tile[0:1, 0:128]
            for _w in range(NWARM_A):
                nc.tensor.matmul(wrm_ps, lhsT=wrm_l, rhs=wrm_r,
                                 start=True, stop=True)
            # style_ps = sum_j fce_j^T wv_j + fb2 (scales pre-folded)
            # style_ps = sum_j fce_j^T wv_j; slot j=JW carries fce_bias
            # via a one-hot wv column (host-packed)
            style_ps = ppsbig[0:CIN, 0:1]
            for j in range(JW + 1):
                nc.tensor.matmul(style_ps, lhsT=fce_sb[:, j, :],
                                 rhs=wv_sb[:, j:j + 1],
                                 start=(j == 0), stop=(j == JW))
            for _w in range(NWARM_B):
                nc.tensor.matmul(wrm_ps, lhsT=wrm_l, rhs=wrm_r,
                                 start=True, stop=True)

            stylec = prep.tile([CIN, 1], f32)
            nc.vector.tensor_copy(stylec, style_ps)
            emit_band(0, 0)

            # Block-structured lhsT tiles, straight from ck * stylec (no
            # demod mul -- applied at drain time).
            # lhsT[k, m]: k<64 = channels of x row XA, k>=64 = x row XA+2;
            # m<64 = out row r (ro=0) channels, m>=64 = out row r+1 (ro=1).
            # mm1 (XA = r-1): (A,ro0)=w[-1,s-1] (B,ro0)=w[+1,s-1]
            #                 (B,ro1)=w[0,s-1]  (A,ro1)=0
            # mm2 (XA = r):   (A,ro0)=w[0,s-1]  (A,ro1)=w[-1,s-1]
            #                 (B,ro1)=w[+1,s-1] (B,ro0)=0
            # wp tap index t = (dy+1)*3 + (dx+1), dx = s-1.
            def lmul(dst, t0):
                nc.vector.tensor_scalar_mul(dst, ck_sb[:, t0:t0 + 3, :],
                                            stylec)

            lmul(L1[0:CIN, :, 0:NF], 0)
            lmul(L1[CIN:2 * CIN, :, 0:NF], 6)
            lmul(L1[CIN:2 * CIN, :, NF:2 * NF], 3)
            lmul(L2[0:CIN, :, 0:NF], 3)
            lmul(L2[0:CIN, :, NF:2 * NF], 0)
            lmul(L2[CIN:2 * CIN, :, NF:2 * NF], 6)

            emit_band(0, 1)
            emit_band(0, 2)

            # ckk[c,n] = sum_t ck^2 on DVE; ssum/sroot/recip/wstd2 are
            # emitted inside the sg2 iteration (see loop) so the PE
            # stream never stalls on them.
            sq = prep.tile([CIN, NT, NF], f32)
            nc.vector.tensor_mul(sq.rearrange("c t n -> c (t n)"),
                                 ck_sb.rearrange("c t n -> c (t n)"),
                                 ck_sb.rearrange("c t n -> c (t n)"))
            t4 = prep.tile([CIN, 4, NF], f32)
            nc.vector.tensor_add(t4.rearrange("c t n -> c (t n)"),
                                 sq[:, 0:4, :].rearrange("c t n -> c (t n)"),
                                 sq[:, 4:8, :].rearrange("c t n -> c (t n)"))
            t2 = prep.tile([CIN, 2, NF], f32)
            nc.vector.tensor_add(t2.rearrange("c t n -> c (t n)"),
                                 t4[:, 0:2, :].rearrange("c t n -> c (t n)"),
                                 t4[:, 2:4, :].rearrange("c t n -> c (t n)"))
            ckk = prep.tile([CIN, NF], f32)
            nc.vector.tensor_add(ckk, t2[:, 0, :], t2[:, 1, :])
            nc.vector.tensor_add(ckk, ckk, sq[:, 8, :])
            stylec2 = prep.tile([CIN, 1], f32)
            nc.vector.tensor_mul(stylec2, stylec, stylec)
            eps_sb = prep.tile([CIN, 1], f32)
            nc.vector.memset(eps_sb, 1e-8)

            emit_band(0, 3)
            emit_band(0, 4)

            ssum_ps = ppsbig[0:CIN, 64:65]
            sroot = prep.tile([CIN, 1], f32)
            wstdT = prep.tile([CIN, 1], f32)
            wstd2 = const.tile([2 * NF, 1], f32)

            def emit_wstd():
                nc.tensor.matmul(ssum_ps, lhsT=ckk, rhs=stylec2,
                                 start=True, stop=True)
                nc.scalar.activation(sroot, ssum_ps,
                                     mybir.ActivationFunctionType.Sqrt,
                                     bias=eps_sb, scale=1.0)
                nc.vector.reciprocal(wstdT, sroot)
                nc.vector.tensor_copy(wstd2[0:NF, :], wstdT)
                nc.vector.tensor_copy(wstd2[NF:2 * NF, :], wstdT)

            def drain(sg, ps, yslice):
                # y = ps*wstd[n] + cb[n]; parity alternates ACT/DVE
                if sg % 2 == 0:
                    nc.scalar.activation(
                        yslice, ps,
                        mybir.ActivationFunctionType.Identity,
                        bias=cb2_sb, scale=wstd2)
                else:
                    nc.vector.tensor_scalar(
                        yslice, ps, wstd2, cb2_sb,
                        op0=mybir.AluOpType.mult,
                        op1=mybir.AluOpType.add)

            # ---- main conv loop (software-pipelined emission) ----
            # Chunk ci+1's segment loads and dup bands are emitted inside
            # chunk ci's supergroup loop so every engine queue interleaves
            # producer work for the next chunk with consumer work for the
            # current one.
            first = True
            for _ in range(repeats):
                if not first:
                    for si in range(nseg(0)):
                        emit_load_seg(0, si)
                    for si in range(nseg(1)):
                        emit_load_seg(1, si)
                    for si in range(nseg(0)):
                        emit_band(0, si)
                pend = []          # (sg, ps, yslice) deferred for sg0..2
                ys = None
                for ci in range(NCH):
                    xx = xxb[ci % 4]
                    xxr = xx.rearrange("p (a b) w -> p b a w", b=2)
                    for q in range(SGC):
                        if q == 0 and ci + 2 < NCH:
                            for si in range(nseg(ci + 2)):
                                emit_load_seg(ci + 2, si)
                        if ci + 1 < NCH and q % 2 == 1:
                            si = (q - 1) // 2
                            if si < nseg(ci + 1):
                                emit_band(ci + 1, si)
                        sg = ci * SGC + q
                        k2 = sg % 2
                        if k2 == 0:
                            ys = yout.tile([2 * NF, 2 * 2 * W], f16)
                        ps = accp.tile([2 * NF, 2 * W], f32)
                        for s in range(KK):
                            nc.tensor.matmul(
                                ps, lhsT=L1[:, s, :],
                                rhs=xxr[:, 0, 2 * q:2 * q + 2, s:s + W],
                                start=(s == 0), stop=False)
                        for s in range(KK):
                            nc.tensor.matmul(
                                ps, lhsT=L2[:, s, :],
                                rhs=xxr[:, 1, 2 * q:2 * q + 2, s:s + W],
                                start=False, stop=(s == KK - 1))
                        yslice = ys[:, k2 * 2 * W:(k2 + 1) * 2 * W]
                        if first and sg < 2:
                            pend.append((sg, ps, yslice, ys))
                            continue
                        if first and sg == 2:
                            emit_wstd()
                            for (psg, p_ps, p_ysl, p_ys) in pend:
                                drain(psg, p_ps, p_ysl)
                            nc.scalar.dma_start(
                                out=ytd[:, 0:4 * W], in_=pend[1][3])
                            pend = []
                        drain(sg, ps, yslice)
                        if sg >= NSG - 2:
                            # last two supergroups store individually so
                            # the final store is as small/early as possible
                            nc.scalar.dma_start(
                                out=ytd[:, sg * 2 * W:(sg + 1) * 2 * W],
                                in_=yslice)
                        elif k2 == 1:
                            nc.scalar.dma_start(
                                out=ytd[:, (sg - 1) * 2 * W:(sg + 1) * 2 * W],
                                in_=ys)
                first = False

    nc.compile()
    return nc


def _get(repeats=1):
    if repeats not in _CACHE:
        _CACHE[repeats] = _build(repeats)
    return _CACHE[repeats]


def _wv_aug(w_b):
    # [128, JW+1]: col JW is the one-hot selecting the bias row
    out = np.zeros((128, JW + 1), np.float32)
    out[:, :JW] = np.asarray(w_b, np.float32).reshape(JW, 128).T
    out[0, JW] = 1.0
    return out


def _fce_aug(fce_kernel, fce_bias):
    # [128, JW+1, CIN]: slots 0..JW-1 = fce_kernel*FCE_SCALE*CONV_SCALE;
    # slot JW partition 0 = fce_bias*CONV_SCALE (selected by the one-hot)
    out = np.zeros((128, JW + 1, CIN), np.float32)
    out[:, :JW, :] = (np.asarray(fce_kernel, np.float32)
                      * (FCE_SCALE * CONV_SCALE)).reshape(
                          JW, 128, CIN).transpose(1, 0, 2)
    out[0, JW, :] = np.asarray(fce_bias, np.float32) * CONV_SCALE
    return np.ascontiguousarray(out)


def _pack(x_b, w_b, fce_kernel, fce_bias, conv_kernel, conv_bias):
    f16 = np.float16
    return {
        "xt": np.ascontiguousarray(
            np.asarray(x_b, np.float32).transpose(2, 0, 1)).astype(f16),
        "wv": _wv_aug(w_b).astype(f16),
        "fce_k": _fce_aug(fce_kernel, fce_bias).astype(f16),
        "ck": np.ascontiguousarray(
            np.asarray(conv_kernel, np.float32)
            .transpose(2, 0, 1, 3).reshape(CIN, KK * KK, NF)).astype(f16),
        "cb": np.asarray(conv_bias, np.float32),
    }


def kernel(x, w, fce_kernel, fce_bias, conv_kernel, conv_bias):
    from concourse.bass_utils import run_bass_kernel_spmd

    nc = _get()
    in_maps = [_pack(x[b], w[b], fce_kernel, fce_bias,
                     conv_kernel, conv_bias) for b in range(B)]
    res = run_bass_kernel_spmd(nc, in_maps, core_ids=list(range(NCORES)))
    out = np.empty((B, H, W, NF), np.float32)
    for b in range(B):
        a = np.asarray(res.results[b]["ytd"]).astype(np.float32)
        # [ro*64+n, g*W+col] -> [h, w, n] with h = 2g + ro
        a = a.reshape(2, NF, H // 2, W).transpose(2, 0, 3, 1)
        out[b] = a.reshape(H, W, NF)
    return out
